# revision 21
# baseline (speedup 1.0000x reference)
"""Trainium2 Bass kernel for causal multi-head attention with QKV/O projections.

Problem: x [1, 2048, 1024] f32, W_qkv [1024, 3072] (q|k|v blocks), W_o
[1024, 1024], H=16 heads, head_dim=64, dense causal attention,
y = softmax(q k^T / 8, causal) v, out = y @ W_o.

Sharding: head-parallel over 8 NeuronCores (2 heads per core). Each core
computes q/k/v projections for its 2 heads, causal attention, and a partial
O-projection (its 128 attention-output columns against its 128 rows of W_o).
The host sums the 8 partial outputs.

On-core dataflow (bf16 into the PE, f32 accumulation in PSUM):
  - xT [D, T] arrives pre-transposed from the host, so projections need no
    on-chip transposes:
       qT/kT [128, T] = W.T @ xT       (2 heads stacked on partitions)
       v     [T, 128] = x @ Wv         (lhsT = xT tiles)
    v is stored with a constant-1 column appended per head ([v_h | 1]), so
    the attention-V matmul also accumulates the softmax denominator.
  - attention is computed transposed: S_T [tk, tq] = kT-tile.T @ qT-tile,
    P_T = exp(S_T/8) in one ACT op per (tk, tq-block) position covering both
    heads (no max subtraction; |S| <= ~4 for this data), causal mask applied
    on diagonal 128x128 blocks; fully-masked blocks skipped and both heads
    column-trimmed on diagonal blocks.
  - numer_T/den: [65, tq] = [v_h | 1].T @ P_T per head. The denominator row
    is reciprocal'd on DVE, partition-broadcast on GpSimd, and one
    elementwise multiply produces the normalized attention output.
  - the normalized numer_T is exactly the O-projection lhsT: y_partial
    [T, D] = att.T.T @ wo_rows, evacuated bf16 and summed on the host.

Scheduling: the PE has a p-state ramp (full 2.4 GHz only after ~3us of
continuous execution), so the emission order keeps the PE dense:
  - the attention i-loop is software-pipelined: S(i+1) is emitted before
    AV(i), so ACT's exp(i) overlaps the PE's S(i+1);
  - projection matmuls for round r+1 (and the O-projection of the previous
    round during the last attention round) are drained quota-wise inside
    the attention i-loop, filling any PE slack;
  - xT is DMA'd in column halves across 4 issuing engines so round-0
    projections start as soon as the first half lands.
"""

from contextlib import ExitStack
from itertools import chain

import numpy as np
import ml_dtypes

import concourse.bacc as bacc
import concourse.mybir as mybir
import concourse.tile as tile

BF16 = ml_dtypes.bfloat16
T = 2048
D = 1024
HD = 64
N_CORES = 8
KD = D // 128          # 8 contraction chunks for projections
NT128 = T // 128       # 16
NT512 = T // 512       # 4
VS = 130               # v_sb per-tile stride: [v_h0(64) | 1 | v_h1(64) | 1]
HALF = T // 2
SCALE = 1.0 / 8.0      # 1/sqrt(64)
WARM_N = 48            # p-state warm-up matmuls while input DMA is in flight

F32 = mybir.dt.float32
BF = mybir.dt.bfloat16

_SENTINEL = object()


class _Work:
    """Wraps an emission generator that yields False mid-segment and True at
    segment boundaries (points where every pool accumulation it opened is
    closed, so other users of the same pools may allocate)."""

    def __init__(self, gen):
        self.gen = gen
        self.at_boundary = True
        self.done = False

    def step(self):
        r = next(self.gen, _SENTINEL)
        if r is _SENTINEL:
            self.done = True
            self.at_boundary = True
            return False
        self.at_boundary = bool(r)
        return True

    def drain_to_boundary(self):
        while not (self.at_boundary or self.done):
            self.step()

    def drain_all(self):
        while not self.done:
            self.step()


def _kernel(tc, y, xT, wq, wk, wv, wo, mask, ident, dbg=None):
    nc = tc.nc
    Exp = mybir.ActivationFunctionType.Exp

    with ExitStack() as ctx:
        persist = ctx.enter_context(tc.tile_pool(name="persist", bufs=1))
        ps_mm = ctx.enter_context(tc.tile_pool(name="ps_mm", bufs=2, space="PSUM"))
        ps_s = ctx.enter_context(tc.tile_pool(name="ps_s", bufs=2, space="PSUM"))
        ps_av = ctx.enter_context(tc.tile_pool(name="ps_av", bufs=1, space="PSUM"))
        pool_p = ctx.enter_context(tc.tile_pool(name="pool_p", bufs=5))
        pool_r = ctx.enter_context(tc.tile_pool(name="pool_r", bufs=2))
        pool_y = ctx.enter_context(tc.tile_pool(name="pool_y", bufs=3))

        wq_sb = persist.tile([128, D], BF, tag="wq")
        wk_sb = persist.tile([128, D], BF, tag="wk")
        wv_sb = persist.tile([128, D], BF, tag="wv")
        wo_sb = persist.tile([128, D], BF, tag="wo")
        mask_sb = persist.tile([128, 128], BF, tag="mask")
        xT_sb = persist.tile([128, KD * T], BF, tag="xT")  # d-chunk d at cols [d*T,(d+1)*T)

        # ---- p-state warm-up: the PE only reaches full clock after ~3us of
        # continuous execution, so stream dummy matmuls on memset scratch
        # while the input DMA is still in flight.
        warm_sb = persist.tile([128, 640], BF, tag="warm")
        nc.vector.memset(warm_sb[:], 0.5)

        # ---- input DMA across 3 issuing engines; xT lands half-0 first so
        # round-0/1 projections can start before half-1 arrives.
        nc.sync.dma_start(wq_sb[:], wq[:])
        nc.gpsimd.dma_start(wk_sb[:], wk[:])
        nc.scalar.dma_start(wv_sb[:], wv[:])
        nc.gpsimd.dma_start(wo_sb[:], wo[:])
        nc.sync.dma_start(mask_sb[:], mask[:])
        ident_sb = persist.tile([128, 128], F32, tag="ident")
        nc.scalar.dma_start(ident_sb[:], ident[:])
        engs = (nc.sync, nc.gpsimd, nc.scalar)
        k = 0
        for h in range(2):
            for d in range(KD):
                engs[k % 3].dma_start(
                    xT_sb[:, d * T + h * HALF: d * T + (h + 1) * HALF],
                    xT[d * 128:(d + 1) * 128, h * HALF:(h + 1) * HALF],
                )
                k += 1

        qT_sb = persist.tile([128, T], BF, tag="qT")   # partitions 0-63 head0, 64-127 head1
        kT_sb = persist.tile([128, T], BF, tag="kT")
        vT_sb = persist.tile([128, T], F32, tag="vT")  # pre-transpose v, f32 so the
        # PE transpose output shares the f32 ps_mm "mm" PSUM slots
        v_sb = persist.tile([128, NT128 * VS], BF, tag="v")
        v_cols = v_sb[:].rearrange("p (t s) -> p t s", s=VS)
        nc.vector.memset(v_cols[:, :, 64:65], 1.0)     # ones columns only
        nc.vector.memset(v_cols[:, :, 129:130], 1.0)
        ones32 = persist.tile([65, HD], F32, tag="ones32")
        nc.vector.memset(ones32[:], 1.0)
        att_sb = persist.tile([128, T], BF, tag="att")  # normalized numer_T

        def proj_gen(r):
            """QKV projections for column-block r: all three are
            weight-stationary (8 LDWEIGHTS each); v additionally transposes
            its 4 128-tiles on the PE (identity stream) into v_sb layout.
            Yields True when the open ps_mm segment has been closed."""
            for w_sb, dst in ((wq_sb, qT_sb), (wk_sb, kT_sb), (wv_sb, vT_sb)):
                ps = ps_mm.tile([128, 512], F32, tag="mm")
                for d in range(KD):
                    nc.tensor.matmul(
                        ps[:],
                        lhsT=w_sb[:, d * 128:(d + 1) * 128],
                        rhs=xT_sb[:, d * T + r * 512: d * T + (r + 1) * 512],
                        start=(d == 0), stop=(d == KD - 1),
                    )
                    yield False
                nc.vector.tensor_copy(dst[:, r * 512:(r + 1) * 512], ps[:])
                yield True
            for t in range(4 * r, 4 * r + 4):
                ps_t = ps_mm.tile([128, 512], F32, tag="mm")
                nc.tensor.transpose(
                    ps_t[:, 0:128], vT_sb[:, t * 128:(t + 1) * 128], ident_sb[:],
                )
                yield False
                dst = v_sb[:, VS * t: VS * t + VS].rearrange(
                    "p (a b) -> p a b", b=65)[:, :, 0:64]
                srcv = ps_t[:, 0:128].rearrange("p (a b) -> p a b", b=64)
                nc.vector.tensor_copy(dst, srcv)
                yield True

        def oproj_gen(tiles):
            """O-projection rows for the given T-chunk indices."""
            for t in tiles:
                y_sb = pool_y.tile([128, 1024], BF, tag="y")
                for nh in range(2):
                    ps = ps_mm.tile([128, 512], F32, tag="mm")
                    nc.tensor.matmul(
                        ps[:],
                        lhsT=att_sb[:, t * 128:(t + 1) * 128],
                        rhs=wo_sb[:, nh * 512:(nh + 1) * 512],
                        start=True, stop=True,
                    )
                    yield False
                    if nh == 0:
                        nc.vector.tensor_copy(y_sb[:, 0:512], ps[:])
                    else:
                        nc.scalar.copy(y_sb[:, 512:1024], ps[:])
                    yield True
                eng = nc.sync if t % 2 == 0 else nc.gpsimd
                eng.dma_start(y[t * 128:(t + 1) * 128, :], y_sb[:])
                yield True

        def attn_round(j, work, quota):
            """Causal attention for tq block j, software-pipelined; drains
            up to `quota` interleave items per i-iteration."""
            n_i = 4 * j + 4
            avden = ps_av.tile([128, 1024], F32, tag="avden")
            s_tiles = [None] * n_i

            def emit_S(i):
                m = i - 4 * j
                off = 128 * m if m > 0 else 0
                s_pair = ps_s.tile([128, 1024], F32, tag="s")
                nc.tensor.matmul(
                    s_pair[:, off:512],
                    lhsT=kT_sb[0:64, i * 128:(i + 1) * 128],
                    rhs=qT_sb[0:64, j * 512 + off:(j + 1) * 512],
                    start=True, stop=True, tile_position=(0, 0),
                )
                nc.tensor.matmul(
                    s_pair[:, 512:1024 - off],
                    lhsT=kT_sb[64:128, i * 128:(i + 1) * 128],
                    rhs=qT_sb[64:128, j * 512 + off:(j + 1) * 512],
                    start=True, stop=True, tile_position=(64, 0),
                )
                s_tiles[i] = s_pair

            emit_S(0)
            for i in range(n_i):
                if i + 1 < n_i:
                    emit_S(i + 1)
                m = i - 4 * j
                off = 128 * m if m > 0 else 0
                ncol = 512 - off
                first, last = (i == 0), (i == n_i - 1)
                s_pair = s_tiles[i]
                s_tiles[i] = None
                p_sb = pool_p.tile([128, 1024], BF, tag="p")
                nc.scalar.activation(
                    p_sb[:, off:512 + ncol], s_pair[:, off:512 + ncol], Exp,
                    scale=SCALE,
                )
                if m >= 0:  # causal mask on the 128x128 diagonal sub-blocks
                    nc.vector.tensor_mul(
                        p_sb[:, off:off + 128], p_sb[:, off:off + 128], mask_sb[:],
                    )
                    nc.vector.tensor_mul(
                        p_sb[:, 512:640], p_sb[:, 512:640], mask_sb[:],
                    )
                nc.tensor.matmul(
                    avden[0:65, off:512],
                    lhsT=v_sb[:, VS * i: VS * i + 65],
                    rhs=p_sb[:, off:512],
                    start=first, stop=last,
                )
                nc.tensor.matmul(
                    avden[0:65, 512 + off:1024],
                    lhsT=v_sb[:, VS * i + 65: VS * i + 130],
                    rhs=p_sb[:, 512:512 + ncol],
                    start=first, stop=last,
                )
                for _ in range(quota):
                    if not work.step():
                        break
            return avden

        def normalize(j, avden):
            """Baseline normalize: K=1 PE matmul broadcasts the denominator
            row across 64 partitions (custom DVE ops require partition base
            0, so a direct partition-shifted reciprocal is not available).
            Batched engine order: DVE copies, PE broadcasts, DVE recip+mul."""
            denrows = []
            for h in range(2):
                denrow = pool_r.tile([65, 512], F32, tag="denrow")
                nc.vector.tensor_copy(
                    denrow[64:65, :], avden[64:65, h * 512:(h + 1) * 512])
                denrows.append(denrow)
            bcs = []
            for h in range(2):
                bc_ps = ps_mm.tile([128, 512], F32, tag="mm")
                nc.tensor.matmul(
                    bc_ps[0:64, :], lhsT=ones32[64:65, :], rhs=denrows[h][64:65, :],
                    start=True, stop=True,
                )
                bcs.append(bc_ps)
            for h in range(2):
                recip = pool_r.tile([64, 512], F32, tag="recip")
                nc.vector.reciprocal_approx_fast(recip[:], bcs[h][0:64, :])
                nc.vector.tensor_mul(
                    att_sb[h * 64:(h + 1) * 64, j * 512:(j + 1) * 512],
                    avden[0:64, h * 512:(h + 1) * 512], recip[:],
                )

        # warm-up stream, then round 0 projections (PE hot when xT lands)
        warm_ps = ps_s.tile([128, 1024], F32, tag="s")
        for _ in range(WARM_N):
            nc.tensor.matmul(
                warm_ps[:, 0:512], lhsT=warm_sb[:, 0:128], rhs=warm_sb[:, 128:640],
                start=True, stop=True,
            )
        for _ in proj_gen(0):
            pass

        # r=0: attn(0) ⟂ proj(1); normalize(0); proj(1) rest; oproj(0) t 0-2
        # r=1: attn(1) ⟂ proj(2); normalize(1); proj(2) rest; oproj(1) t 4-6
        # r=2: attn(2) ⟂ proj(3); normalize(2); proj(3) rest
        # r=3: attn(3) ⟂ oproj(2) t 8-10; normalize(3); deferred t 3,7,11
        #      (ready att data keeps the PE busy under normalize(3)); oproj(3)
        quotas = (8, 4, 3, 1)
        work = _Work(proj_gen(1))
        for r in range(NT512):
            avden = attn_round(r, work, quotas[r])
            work.drain_to_boundary()
            if r == 3:
                # deferred tiles have long-ready att data: they keep the PE
                # busy while normalize(3)'s DVE chain runs
                _Work(oproj_gen([3, 7])).drain_all()
            normalize(r, avden)
            work.drain_all()
            if r < 2:
                _Work(oproj_gen([4 * r, 4 * r + 1, 4 * r + 2])).drain_all()
                work = _Work(proj_gen(r + 2))
            elif r == 2:
                work = _Work(oproj_gen([8, 9, 10]))
            else:
                _Work(oproj_gen([11, 12, 13, 14, 15])).drain_all()

        if dbg is not None:
            for name, sb in (("qT", qT_sb), ("kT", kT_sb), ("att", att_sb)):
                nc.sync.dma_start(dbg[name][:], sb[:])


def _build_program(debug_dumps=False):
    nc = bacc.Bacc("TRN2", debug=False, num_devices=N_CORES)
    xT = nc.dram_tensor("xT", [D, T], BF, kind="ExternalInput").ap()
    wq = nc.dram_tensor("wq", [128, D], BF, kind="ExternalInput").ap()
    wk = nc.dram_tensor("wk", [128, D], BF, kind="ExternalInput").ap()
    wv = nc.dram_tensor("wv", [128, D], BF, kind="ExternalInput").ap()
    wo = nc.dram_tensor("wo", [128, D], BF, kind="ExternalInput").ap()
    mask = nc.dram_tensor("mask", [128, 128], BF, kind="ExternalInput").ap()
    ident = nc.dram_tensor("ident", [128, 128], F32, kind="ExternalInput").ap()
    y = nc.dram_tensor("y", [T, D], BF, kind="ExternalOutput").ap()
    dbg = None
    if debug_dumps:
        dbg = {
            name: nc.dram_tensor(f"dbg_{name}", [128, T], BF, kind="ExternalOutput").ap()
            for name in ("qT", "kT", "att")
        }

    with tile.TileContext(nc) as tc:
        _kernel(tc, y, xT, wq, wk, wv, wo, mask, ident, dbg=dbg)
    nc.compile()
    return nc


_NC = None


def _get_program():
    global _NC
    if _NC is None:
        _NC = _build_program()
    return _NC


def _rearrange_w(w_cols):
    """[1024, 128] f32 slice of W_qkv -> [128, 1024] bf16 with d-chunk d at
    cols [d*128, (d+1)*128): out[p, d*128 + m] = w_cols[d*128 + p, m]."""
    return np.ascontiguousarray(
        w_cols.reshape(KD, 128, 128).transpose(1, 0, 2).reshape(128, KD * 128)
    ).astype(BF16)


def make_in_maps(x, W_qkv, W_o):
    x2 = np.asarray(x, dtype=np.float32).reshape(T, D)
    W_qkv = np.asarray(W_qkv, dtype=np.float32)
    W_o = np.asarray(W_o, dtype=np.float32)

    xT_bf = np.ascontiguousarray(x2.T).astype(BF16)
    mask = np.triu(np.ones((128, 128), dtype=np.float32)).astype(BF16)
    ident = np.eye(128, dtype=np.float32)

    in_maps = []
    for c in range(N_CORES):
        cs = slice(2 * c * HD, 2 * c * HD + 128)
        in_maps.append({
            "xT": xT_bf,
            "wq": _rearrange_w(W_qkv[:, 0 * D:1 * D][:, cs]),
            "wk": _rearrange_w(W_qkv[:, 1 * D:2 * D][:, cs]),
            "wv": _rearrange_w(W_qkv[:, 2 * D:3 * D][:, cs]),
            "wo": np.ascontiguousarray(W_o[c * 128:(c + 1) * 128, :]).astype(BF16),
            "mask": mask,
            "ident": ident,
        })
    return in_maps


def combine_outputs(results):
    y_full = np.zeros((T, D), dtype=np.float32)
    for c in range(N_CORES):
        y_full += results[c]["y"].astype(np.float32)
    return y_full.reshape(1, T, D)


def kernel(x, W_qkv, W_o):
    from concourse.bass_utils import run_bass_kernel_spmd

    nc = _get_program()
    in_maps = make_in_maps(x, W_qkv, W_o)
    res = run_bass_kernel_spmd(nc, in_maps, core_ids=list(range(N_CORES)))
    return combine_outputs(res.results)


# revision 22
# speedup vs baseline: 1.0002x; 1.0002x over previous
"""Trainium2 Bass kernel for causal multi-head attention with QKV/O projections.

Problem: x [1, 2048, 1024] f32, W_qkv [1024, 3072] (q|k|v blocks), W_o
[1024, 1024], H=16 heads, head_dim=64, dense causal attention,
y = softmax(q k^T / 8, causal) v, out = y @ W_o.

Sharding: head-parallel over 8 NeuronCores (2 heads per core). Each core
computes q/k/v projections for its 2 heads, causal attention, and a partial
O-projection (its 128 attention-output columns against its 128 rows of W_o).
The host sums the 8 partial outputs.

On-core dataflow (bf16 into the PE, f32 accumulation in PSUM):
  - xT [D, T] arrives pre-transposed from the host, so projections need no
    on-chip transposes:
       qT/kT [128, T] = W.T @ xT       (2 heads stacked on partitions)
       v     [T, 128] = x @ Wv         (lhsT = xT tiles)
    v is stored with a constant-1 column appended per head ([v_h | 1]), so
    the attention-V matmul also accumulates the softmax denominator.
  - attention is computed transposed: S_T [tk, tq] = kT-tile.T @ qT-tile,
    P_T = exp(S_T/8) in one ACT op per (tk, tq-block) position covering both
    heads (no max subtraction; |S| <= ~4 for this data), causal mask applied
    on diagonal 128x128 blocks; fully-masked blocks skipped and both heads
    column-trimmed on diagonal blocks.
  - numer_T/den: [65, tq] = [v_h | 1].T @ P_T per head. The denominator row
    is reciprocal'd on DVE, partition-broadcast on GpSimd, and one
    elementwise multiply produces the normalized attention output.
  - the normalized numer_T is exactly the O-projection lhsT: y_partial
    [T, D] = att.T.T @ wo_rows, evacuated bf16 and summed on the host.

Scheduling: the PE has a p-state ramp (full 2.4 GHz only after ~3us of
continuous execution), so the emission order keeps the PE dense:
  - the attention i-loop is software-pipelined: S(i+1) is emitted before
    AV(i), so ACT's exp(i) overlaps the PE's S(i+1);
  - projection matmuls for round r+1 (and the O-projection of the previous
    round during the last attention round) are drained quota-wise inside
    the attention i-loop, filling any PE slack;
  - xT is DMA'd in column halves across 4 issuing engines so round-0
    projections start as soon as the first half lands.
"""

from contextlib import ExitStack
from itertools import chain

import numpy as np
import ml_dtypes

import concourse.bacc as bacc
import concourse.mybir as mybir
import concourse.tile as tile

BF16 = ml_dtypes.bfloat16
T = 2048
D = 1024
HD = 64
N_CORES = 8
KD = D // 128          # 8 contraction chunks for projections
NT128 = T // 128       # 16
NT512 = T // 512       # 4
VS = 130               # v_sb per-tile stride: [v_h0(64) | 1 | v_h1(64) | 1]
HALF = T // 2
SCALE = 1.0 / 8.0      # 1/sqrt(64)
WARM_N = 48            # p-state warm-up matmuls while input DMA is in flight

F32 = mybir.dt.float32
BF = mybir.dt.bfloat16

_SENTINEL = object()


class _Work:
    """Wraps an emission generator that yields False mid-segment and True at
    segment boundaries (points where every pool accumulation it opened is
    closed, so other users of the same pools may allocate)."""

    def __init__(self, gen):
        self.gen = gen
        self.at_boundary = True
        self.done = False

    def step(self):
        r = next(self.gen, _SENTINEL)
        if r is _SENTINEL:
            self.done = True
            self.at_boundary = True
            return False
        self.at_boundary = bool(r)
        return True

    def drain_to_boundary(self):
        while not (self.at_boundary or self.done):
            self.step()

    def drain_all(self):
        while not self.done:
            self.step()


def _kernel(tc, y, xT, wq, wk, wv, wo, mask, ident, dbg=None):
    nc = tc.nc
    Exp = mybir.ActivationFunctionType.Exp

    with ExitStack() as ctx:
        persist = ctx.enter_context(tc.tile_pool(name="persist", bufs=1))
        ps_mm = ctx.enter_context(tc.tile_pool(name="ps_mm", bufs=2, space="PSUM"))
        ps_s = ctx.enter_context(tc.tile_pool(name="ps_s", bufs=2, space="PSUM"))
        ps_av = ctx.enter_context(tc.tile_pool(name="ps_av", bufs=1, space="PSUM"))
        pool_p = ctx.enter_context(tc.tile_pool(name="pool_p", bufs=5))
        pool_r = ctx.enter_context(tc.tile_pool(name="pool_r", bufs=2))
        pool_y = ctx.enter_context(tc.tile_pool(name="pool_y", bufs=3))

        wq_sb = persist.tile([128, D], BF, tag="wq")
        wk_sb = persist.tile([128, D], BF, tag="wk")
        wv_sb = persist.tile([128, D], BF, tag="wv")
        wo_sb = persist.tile([128, D], BF, tag="wo")
        mask_sb = persist.tile([128, 128], BF, tag="mask")
        xT_sb = persist.tile([128, KD * T], BF, tag="xT")  # d-chunk d at cols [d*T,(d+1)*T)

        # ---- p-state warm-up: the PE only reaches full clock after ~3us of
        # continuous execution, so stream dummy matmuls on memset scratch
        # while the input DMA is still in flight.
        warm_sb = persist.tile([128, 640], BF, tag="warm")
        nc.vector.memset(warm_sb[:], 0.5)

        # ---- input DMA across 3 issuing engines; xT lands half-0 first so
        # round-0/1 projections can start before half-1 arrives.
        nc.sync.dma_start(wq_sb[:], wq[:])
        nc.gpsimd.dma_start(wk_sb[:], wk[:])
        nc.scalar.dma_start(wv_sb[:], wv[:])
        nc.gpsimd.dma_start(wo_sb[:], wo[:])
        nc.sync.dma_start(mask_sb[:], mask[:])
        ident_sb = persist.tile([128, 128], BF, tag="ident")
        nc.scalar.dma_start(ident_sb[:], ident[:])
        engs = (nc.sync, nc.gpsimd, nc.scalar)
        k = 0
        for h in range(2):
            for d in range(KD):
                engs[k % 3].dma_start(
                    xT_sb[:, d * T + h * HALF: d * T + (h + 1) * HALF],
                    xT[d * 128:(d + 1) * 128, h * HALF:(h + 1) * HALF],
                )
                k += 1

        qT_sb = persist.tile([128, T], BF, tag="qT")   # partitions 0-63 head0, 64-127 head1
        kT_sb = persist.tile([128, T], BF, tag="kT")
        vT_sb = persist.tile([128, T], BF, tag="vT")   # pre-transpose v (like qT/kT)
        v_sb = persist.tile([128, NT128 * VS], BF, tag="v")
        v_cols = v_sb[:].rearrange("p (t s) -> p t s", s=VS)
        nc.vector.memset(v_cols[:, :, 64:65], 1.0)     # ones columns only
        nc.vector.memset(v_cols[:, :, 129:130], 1.0)
        ones32 = persist.tile([65, HD], F32, tag="ones32")
        nc.vector.memset(ones32[:], 1.0)
        att_sb = persist.tile([128, T], BF, tag="att")  # normalized numer_T

        def proj_gen(r):
            """QKV projections for column-block r: all three are
            weight-stationary (8 LDWEIGHTS each); v additionally transposes
            its 4 128-tiles on the PE (identity stream) into v_sb layout.
            Yields True when the open ps_mm segment has been closed."""
            for w_sb, dst in ((wq_sb, qT_sb), (wk_sb, kT_sb), (wv_sb, vT_sb)):
                ps = ps_mm.tile([128, 512], F32, tag="mm")
                for d in range(KD):
                    nc.tensor.matmul(
                        ps[:],
                        lhsT=w_sb[:, d * 128:(d + 1) * 128],
                        rhs=xT_sb[:, d * T + r * 512: d * T + (r + 1) * 512],
                        start=(d == 0), stop=(d == KD - 1),
                    )
                    yield False
                nc.vector.tensor_copy(dst[:, r * 512:(r + 1) * 512], ps[:])
                yield True
            for t in range(4 * r, 4 * r + 4):
                ps_t = ps_mm.tile([128, 512], F32, tag="mm")
                # exact transpose as a plain bf16 matmul against the identity
                nc.tensor.matmul(
                    ps_t[:, 0:128],
                    lhsT=vT_sb[:, t * 128:(t + 1) * 128], rhs=ident_sb[:],
                    start=True, stop=True,
                )
                yield False
                dst = v_sb[:, VS * t: VS * t + VS].rearrange(
                    "p (a b) -> p a b", b=65)[:, :, 0:64]
                srcv = ps_t[:, 0:128].rearrange("p (a b) -> p a b", b=64)
                nc.vector.tensor_copy(dst, srcv)
                yield True

        def oproj_gen(tiles):
            """O-projection rows for the given T-chunk indices."""
            for t in tiles:
                y_sb = pool_y.tile([128, 1024], BF, tag="y")
                for nh in range(2):
                    ps = ps_mm.tile([128, 512], F32, tag="mm")
                    nc.tensor.matmul(
                        ps[:],
                        lhsT=att_sb[:, t * 128:(t + 1) * 128],
                        rhs=wo_sb[:, nh * 512:(nh + 1) * 512],
                        start=True, stop=True,
                    )
                    yield False
                    if nh == 0:
                        nc.vector.tensor_copy(y_sb[:, 0:512], ps[:])
                    else:
                        nc.scalar.copy(y_sb[:, 512:1024], ps[:])
                    yield True
                eng = nc.sync if t % 2 == 0 else nc.gpsimd
                eng.dma_start(y[t * 128:(t + 1) * 128, :], y_sb[:])
                yield True

        def attn_round(j, work, quota):
            """Causal attention for tq block j, software-pipelined; drains
            up to `quota` interleave items per i-iteration."""
            n_i = 4 * j + 4
            avden = ps_av.tile([128, 1024], F32, tag="avden")
            s_tiles = [None] * n_i

            def emit_S(i):
                m = i - 4 * j
                off = 128 * m if m > 0 else 0
                s_pair = ps_s.tile([128, 1024], F32, tag="s")
                nc.tensor.matmul(
                    s_pair[:, off:512],
                    lhsT=kT_sb[0:64, i * 128:(i + 1) * 128],
                    rhs=qT_sb[0:64, j * 512 + off:(j + 1) * 512],
                    start=True, stop=True, tile_position=(0, 0),
                )
                nc.tensor.matmul(
                    s_pair[:, 512:1024 - off],
                    lhsT=kT_sb[64:128, i * 128:(i + 1) * 128],
                    rhs=qT_sb[64:128, j * 512 + off:(j + 1) * 512],
                    start=True, stop=True, tile_position=(64, 0),
                )
                s_tiles[i] = s_pair

            emit_S(0)
            for i in range(n_i):
                if i + 1 < n_i:
                    emit_S(i + 1)
                m = i - 4 * j
                off = 128 * m if m > 0 else 0
                ncol = 512 - off
                first, last = (i == 0), (i == n_i - 1)
                s_pair = s_tiles[i]
                s_tiles[i] = None
                p_sb = pool_p.tile([128, 1024], BF, tag="p")
                nc.scalar.activation(
                    p_sb[:, off:512 + ncol], s_pair[:, off:512 + ncol], Exp,
                    scale=SCALE,
                )
                if m >= 0:  # causal mask on the 128x128 diagonal sub-blocks
                    nc.vector.tensor_mul(
                        p_sb[:, off:off + 128], p_sb[:, off:off + 128], mask_sb[:],
                    )
                    nc.vector.tensor_mul(
                        p_sb[:, 512:640], p_sb[:, 512:640], mask_sb[:],
                    )
                nc.tensor.matmul(
                    avden[0:65, off:512],
                    lhsT=v_sb[:, VS * i: VS * i + 65],
                    rhs=p_sb[:, off:512],
                    start=first, stop=last,
                )
                nc.tensor.matmul(
                    avden[0:65, 512 + off:1024],
                    lhsT=v_sb[:, VS * i + 65: VS * i + 130],
                    rhs=p_sb[:, 512:512 + ncol],
                    start=first, stop=last,
                )
                for _ in range(quota):
                    if not work.step():
                        break
            return avden

        def normalize(j, avden):
            """Baseline normalize: K=1 PE matmul broadcasts the denominator
            row across 64 partitions (custom DVE ops require partition base
            0, so a direct partition-shifted reciprocal is not available).
            Batched engine order: DVE copies, PE broadcasts, DVE recip+mul."""
            denrows = []
            for h in range(2):
                denrow = pool_r.tile([65, 512], F32, tag="denrow")
                nc.vector.tensor_copy(
                    denrow[64:65, :], avden[64:65, h * 512:(h + 1) * 512])
                denrows.append(denrow)
            bcs = []
            for h in range(2):
                bc_ps = ps_mm.tile([128, 512], F32, tag="mm")
                nc.tensor.matmul(
                    bc_ps[0:64, :], lhsT=ones32[64:65, :], rhs=denrows[h][64:65, :],
                    start=True, stop=True,
                )
                bcs.append(bc_ps)
            for h in range(2):
                recip = pool_r.tile([64, 512], F32, tag="recip")
                nc.vector.reciprocal_approx_fast(recip[:], bcs[h][0:64, :])
                nc.vector.tensor_mul(
                    att_sb[h * 64:(h + 1) * 64, j * 512:(j + 1) * 512],
                    avden[0:64, h * 512:(h + 1) * 512], recip[:],
                )

        # warm-up stream, then round 0 projections (PE hot when xT lands)
        warm_ps = ps_s.tile([128, 1024], F32, tag="s")
        for _ in range(WARM_N):
            nc.tensor.matmul(
                warm_ps[:, 0:512], lhsT=warm_sb[:, 0:128], rhs=warm_sb[:, 128:640],
                start=True, stop=True,
            )
        for _ in proj_gen(0):
            pass

        # r=0: attn(0) ⟂ proj(1); normalize(0); proj(1) rest; oproj(0) t 0-2
        # r=1: attn(1) ⟂ proj(2); normalize(1); proj(2) rest; oproj(1) t 4-6
        # r=2: attn(2) ⟂ proj(3); normalize(2); proj(3) rest
        # r=3: attn(3) ⟂ oproj(2) t 8-10; normalize(3); deferred t 3,7,11
        #      (ready att data keeps the PE busy under normalize(3)); oproj(3)
        quotas = (8, 4, 3, 1)
        work = _Work(proj_gen(1))
        for r in range(NT512):
            avden = attn_round(r, work, quotas[r])
            work.drain_to_boundary()
            if r == 3:
                # deferred tiles have long-ready att data: they keep the PE
                # busy while normalize(3)'s DVE chain runs
                _Work(oproj_gen([3, 7])).drain_all()
            normalize(r, avden)
            work.drain_all()
            if r < 2:
                _Work(oproj_gen([4 * r, 4 * r + 1, 4 * r + 2])).drain_all()
                work = _Work(proj_gen(r + 2))
            elif r == 2:
                work = _Work(oproj_gen([8, 9, 10]))
            else:
                _Work(oproj_gen([11, 12, 13, 14, 15])).drain_all()

        if dbg is not None:
            for name, sb in (("qT", qT_sb), ("kT", kT_sb), ("att", att_sb)):
                nc.sync.dma_start(dbg[name][:], sb[:])


def _build_program(debug_dumps=False):
    nc = bacc.Bacc("TRN2", debug=False, num_devices=N_CORES)
    xT = nc.dram_tensor("xT", [D, T], BF, kind="ExternalInput").ap()
    wq = nc.dram_tensor("wq", [128, D], BF, kind="ExternalInput").ap()
    wk = nc.dram_tensor("wk", [128, D], BF, kind="ExternalInput").ap()
    wv = nc.dram_tensor("wv", [128, D], BF, kind="ExternalInput").ap()
    wo = nc.dram_tensor("wo", [128, D], BF, kind="ExternalInput").ap()
    mask = nc.dram_tensor("mask", [128, 128], BF, kind="ExternalInput").ap()
    ident = nc.dram_tensor("ident", [128, 128], BF, kind="ExternalInput").ap()
    y = nc.dram_tensor("y", [T, D], BF, kind="ExternalOutput").ap()
    dbg = None
    if debug_dumps:
        dbg = {
            name: nc.dram_tensor(f"dbg_{name}", [128, T], BF, kind="ExternalOutput").ap()
            for name in ("qT", "kT", "att")
        }

    with tile.TileContext(nc) as tc:
        _kernel(tc, y, xT, wq, wk, wv, wo, mask, ident, dbg=dbg)
    nc.compile()
    return nc


_NC = None


def _get_program():
    global _NC
    if _NC is None:
        _NC = _build_program()
    return _NC


def _rearrange_w(w_cols):
    """[1024, 128] f32 slice of W_qkv -> [128, 1024] bf16 with d-chunk d at
    cols [d*128, (d+1)*128): out[p, d*128 + m] = w_cols[d*128 + p, m]."""
    return np.ascontiguousarray(
        w_cols.reshape(KD, 128, 128).transpose(1, 0, 2).reshape(128, KD * 128)
    ).astype(BF16)


def make_in_maps(x, W_qkv, W_o):
    x2 = np.asarray(x, dtype=np.float32).reshape(T, D)
    W_qkv = np.asarray(W_qkv, dtype=np.float32)
    W_o = np.asarray(W_o, dtype=np.float32)

    xT_bf = np.ascontiguousarray(x2.T).astype(BF16)
    mask = np.triu(np.ones((128, 128), dtype=np.float32)).astype(BF16)
    ident = np.eye(128, dtype=np.float32).astype(BF16)

    in_maps = []
    for c in range(N_CORES):
        cs = slice(2 * c * HD, 2 * c * HD + 128)
        in_maps.append({
            "xT": xT_bf,
            "wq": _rearrange_w(W_qkv[:, 0 * D:1 * D][:, cs]),
            "wk": _rearrange_w(W_qkv[:, 1 * D:2 * D][:, cs]),
            "wv": _rearrange_w(W_qkv[:, 2 * D:3 * D][:, cs]),
            "wo": np.ascontiguousarray(W_o[c * 128:(c + 1) * 128, :]).astype(BF16),
            "mask": mask,
            "ident": ident,
        })
    return in_maps


def combine_outputs(results):
    y_full = np.zeros((T, D), dtype=np.float32)
    for c in range(N_CORES):
        y_full += results[c]["y"].astype(np.float32)
    return y_full.reshape(1, T, D)


def kernel(x, W_qkv, W_o):
    from concourse.bass_utils import run_bass_kernel_spmd

    nc = _get_program()
    in_maps = make_in_maps(x, W_qkv, W_o)
    res = run_bass_kernel_spmd(nc, in_maps, core_ids=list(range(N_CORES)))
    return combine_outputs(res.results)


# revision 23
# speedup vs baseline: 1.0250x; 1.0248x over previous
"""Trainium2 Bass kernel for causal multi-head attention with QKV/O projections.

Problem: x [1, 2048, 1024] f32, W_qkv [1024, 3072] (q|k|v blocks), W_o
[1024, 1024], H=16 heads, head_dim=64, dense causal attention,
y = softmax(q k^T / 8, causal) v, out = y @ W_o.

Sharding: head-parallel over 8 NeuronCores (2 heads per core). Each core
computes q/k/v projections for its 2 heads, causal attention, and a partial
O-projection (its 128 attention-output columns against its 128 rows of W_o).
The host sums the 8 partial outputs.

On-core dataflow (bf16 into the PE, f32 accumulation in PSUM):
  - xT [D, T] arrives pre-transposed from the host, so projections need no
    on-chip transposes:
       qT/kT [128, T] = W.T @ xT       (2 heads stacked on partitions)
       v     [T, 128] = x @ Wv         (lhsT = xT tiles)
    v is stored with a constant-1 column appended per head ([v_h | 1]), so
    the attention-V matmul also accumulates the softmax denominator.
  - attention is computed transposed: S_T [tk, tq] = kT-tile.T @ qT-tile,
    P_T = exp(S_T/8) in one ACT op per (tk, tq-block) position covering both
    heads (no max subtraction; |S| <= ~4 for this data), causal mask applied
    on diagonal 128x128 blocks; fully-masked blocks skipped and both heads
    column-trimmed on diagonal blocks.
  - numer_T/den: [65, tq] = [v_h | 1].T @ P_T per head. The denominator row
    is reciprocal'd on DVE, partition-broadcast on GpSimd, and one
    elementwise multiply produces the normalized attention output.
  - the normalized numer_T is exactly the O-projection lhsT: y_partial
    [T, D] = att.T.T @ wo_rows, evacuated bf16 and summed on the host.

Scheduling: the PE has a p-state ramp (full 2.4 GHz only after ~3us of
continuous execution), so the emission order keeps the PE dense:
  - the attention i-loop is software-pipelined: S(i+1) is emitted before
    AV(i), so ACT's exp(i) overlaps the PE's S(i+1);
  - projection matmuls for round r+1 (and the O-projection of the previous
    round during the last attention round) are drained quota-wise inside
    the attention i-loop, filling any PE slack;
  - xT is DMA'd in column halves across 4 issuing engines so round-0
    projections start as soon as the first half lands.
"""

from contextlib import ExitStack
from itertools import chain

import numpy as np
import ml_dtypes

import concourse.bacc as bacc
import concourse.mybir as mybir
import concourse.tile as tile

BF16 = ml_dtypes.bfloat16
T = 2048
D = 1024
HD = 64
N_CORES = 8
KD = D // 128          # 8 contraction chunks for projections
NT128 = T // 128       # 16
NT512 = T // 512       # 4
VS = 130               # v_sb per-tile stride: [v_h0(64) | 1 | v_h1(64) | 1]
HALF = T // 2
SCALE = 1.0 / 8.0      # 1/sqrt(64)
WARM_N = 48            # p-state warm-up matmuls while input DMA is in flight

F32 = mybir.dt.float32
BF = mybir.dt.bfloat16

_SENTINEL = object()


class _Work:
    """Wraps an emission generator that yields False mid-segment and True at
    segment boundaries (points where every pool accumulation it opened is
    closed, so other users of the same pools may allocate)."""

    def __init__(self, gen):
        self.gen = gen
        self.at_boundary = True
        self.done = False

    def step(self):
        r = next(self.gen, _SENTINEL)
        if r is _SENTINEL:
            self.done = True
            self.at_boundary = True
            return False
        self.at_boundary = bool(r)
        return True

    def drain_to_boundary(self):
        while not (self.at_boundary or self.done):
            self.step()

    def drain_all(self):
        while not self.done:
            self.step()


def _kernel(tc, y, xT, wq, wk, wv, wo, mask, dbg=None):
    nc = tc.nc
    Exp = mybir.ActivationFunctionType.Exp

    with ExitStack() as ctx:
        persist = ctx.enter_context(tc.tile_pool(name="persist", bufs=1))
        ps_mm = ctx.enter_context(tc.tile_pool(name="ps_mm", bufs=2, space="PSUM"))
        ps_s = ctx.enter_context(tc.tile_pool(name="ps_s", bufs=2, space="PSUM"))
        ps_av = ctx.enter_context(tc.tile_pool(name="ps_av", bufs=1, space="PSUM"))
        pool_p = ctx.enter_context(tc.tile_pool(name="pool_p", bufs=5))
        pool_r = ctx.enter_context(tc.tile_pool(name="pool_r", bufs=2))
        pool_y = ctx.enter_context(tc.tile_pool(name="pool_y", bufs=3))

        wq_sb = persist.tile([128, D], BF, tag="wq")
        wk_sb = persist.tile([128, D], BF, tag="wk")
        wv_sb = persist.tile([128, D], BF, tag="wv")
        wo_sb = persist.tile([128, D], BF, tag="wo")
        mask_sb = persist.tile([128, 128], BF, tag="mask")
        xT_sb = persist.tile([128, KD * T], BF, tag="xT")  # d-chunk d at cols [d*T,(d+1)*T)

        # ---- p-state warm-up: the PE only reaches full clock after ~3us of
        # continuous execution, so stream dummy matmuls on memset scratch
        # while the input DMA is still in flight.
        warm_sb = persist.tile([128, 640], BF, tag="warm")
        nc.vector.memset(warm_sb[:], 0.5)

        # ---- input DMA across 3 issuing engines; xT lands half-0 first so
        # round-0/1 projections can start before half-1 arrives.
        nc.sync.dma_start(wq_sb[:], wq[:])
        nc.gpsimd.dma_start(wk_sb[:], wk[:])
        nc.scalar.dma_start(wv_sb[:], wv[:])
        nc.gpsimd.dma_start(wo_sb[:], wo[:])
        nc.sync.dma_start(mask_sb[:], mask[:])
        engs = (nc.sync, nc.gpsimd, nc.scalar)
        k = 0
        for h in range(2):
            for d in range(KD):
                engs[k % 3].dma_start(
                    xT_sb[:, d * T + h * HALF: d * T + (h + 1) * HALF],
                    xT[d * 128:(d + 1) * 128, h * HALF:(h + 1) * HALF],
                )
                k += 1

        qT_sb = persist.tile([128, T], BF, tag="qT")   # partitions 0-63 head0, 64-127 head1
        kT_sb = persist.tile([128, T], BF, tag="kT")
        v_sb = persist.tile([128, NT128 * VS], BF, tag="v")
        v_cols = v_sb[:].rearrange("p (t s) -> p t s", s=VS)
        nc.vector.memset(v_cols[:, :, 64:65], 1.0)     # ones columns only
        nc.vector.memset(v_cols[:, :, 129:130], 1.0)
        ones32 = persist.tile([65, HD], F32, tag="ones32")
        nc.vector.memset(ones32[:], 1.0)
        att_sb = persist.tile([128, T], BF, tag="att")  # normalized numer_T

        def proj_gen(r):
            """QKV projections for column-block r: 48 matmuls + 6 casts.
            Yields True when the open ps_mm segment has been closed."""
            for w_sb, dst in ((wq_sb, qT_sb), (wk_sb, kT_sb)):
                ps = ps_mm.tile([128, 512], F32, tag="mm")
                for d in range(KD):
                    nc.tensor.matmul(
                        ps[:],
                        lhsT=w_sb[:, d * 128:(d + 1) * 128],
                        rhs=xT_sb[:, d * T + r * 512: d * T + (r + 1) * 512],
                        start=(d == 0), stop=(d == KD - 1),
                    )
                    yield False
                nc.vector.tensor_copy(dst[:, r * 512:(r + 1) * 512], ps[:])
                yield True
            for t in range(4 * r, 4 * r + 4):
                ps = ps_mm.tile([128, 512], F32, tag="mm")
                for d in range(KD):
                    nc.tensor.matmul(
                        ps[:, 0:128],
                        lhsT=xT_sb[:, d * T + t * 128: d * T + (t + 1) * 128],
                        rhs=wv_sb[:, d * 128:(d + 1) * 128],
                        start=(d == 0), stop=(d == KD - 1),
                    )
                    yield False
                dst = v_sb[:, VS * t: VS * t + VS].rearrange(
                    "p (a b) -> p a b", b=65)[:, :, 0:64]
                srcv = ps[:, 0:128].rearrange("p (a b) -> p a b", b=64)
                nc.vector.tensor_copy(dst, srcv)
                yield True

        def oproj_gen(tiles):
            """O-projection rows for the given T-chunk indices."""
            for t in tiles:
                y_sb = pool_y.tile([128, 1024], BF, tag="y")
                for nh in range(2):
                    ps = ps_mm.tile([128, 512], F32, tag="mm")
                    nc.tensor.matmul(
                        ps[:],
                        lhsT=att_sb[:, t * 128:(t + 1) * 128],
                        rhs=wo_sb[:, nh * 512:(nh + 1) * 512],
                        start=True, stop=True,
                    )
                    yield False
                    if nh == 0:
                        nc.vector.tensor_copy(y_sb[:, 0:512], ps[:])
                    else:
                        nc.scalar.copy(y_sb[:, 512:1024], ps[:])
                    yield True
                eng = nc.sync if t % 2 == 0 else nc.gpsimd
                eng.dma_start(y[t * 128:(t + 1) * 128, :], y_sb[:])
                yield True

        def attn_round(j, work, quota):
            """Causal attention for tq block j, software-pipelined; drains
            up to `quota` interleave items per i-iteration."""
            n_i = 4 * j + 4
            avden = ps_av.tile([128, 1024], F32, tag="avden")
            s_tiles = [None] * n_i

            def emit_S(i):
                m = i - 4 * j
                off = 128 * m if m > 0 else 0
                s_pair = ps_s.tile([128, 1024], F32, tag="s")
                nc.tensor.matmul(
                    s_pair[:, off:512],
                    lhsT=kT_sb[0:64, i * 128:(i + 1) * 128],
                    rhs=qT_sb[0:64, j * 512 + off:(j + 1) * 512],
                    start=True, stop=True, tile_position=(0, 0),
                )
                nc.tensor.matmul(
                    s_pair[:, 512:1024 - off],
                    lhsT=kT_sb[64:128, i * 128:(i + 1) * 128],
                    rhs=qT_sb[64:128, j * 512 + off:(j + 1) * 512],
                    start=True, stop=True, tile_position=(64, 0),
                )
                s_tiles[i] = s_pair

            emit_S(0)
            for i in range(n_i):
                if i + 1 < n_i:
                    emit_S(i + 1)
                m = i - 4 * j
                off = 128 * m if m > 0 else 0
                ncol = 512 - off
                first, last = (i == 0), (i == n_i - 1)
                s_pair = s_tiles[i]
                s_tiles[i] = None
                p_sb = pool_p.tile([128, 1024], BF, tag="p")
                nc.scalar.activation(
                    p_sb[:, off:512 + ncol], s_pair[:, off:512 + ncol], Exp,
                    scale=SCALE,
                )
                if m >= 0:  # causal mask on the 128x128 diagonal sub-blocks
                    nc.vector.tensor_mul(
                        p_sb[:, off:off + 128], p_sb[:, off:off + 128], mask_sb[:],
                    )
                    nc.vector.tensor_mul(
                        p_sb[:, 512:640], p_sb[:, 512:640], mask_sb[:],
                    )
                nc.tensor.matmul(
                    avden[0:65, off:512],
                    lhsT=v_sb[:, VS * i: VS * i + 65],
                    rhs=p_sb[:, off:512],
                    start=first, stop=last,
                )
                nc.tensor.matmul(
                    avden[0:65, 512 + off:1024],
                    lhsT=v_sb[:, VS * i + 65: VS * i + 130],
                    rhs=p_sb[:, 512:512 + ncol],
                    start=first, stop=last,
                )
                for _ in range(quota):
                    if not work.step():
                        break
            return avden

        def normalize(j, avden):
            """Baseline normalize: K=1 PE matmul broadcasts the denominator
            row across 64 partitions (custom DVE ops require partition base
            0, so a direct partition-shifted reciprocal is not available).
            Batched engine order: DVE copies, PE broadcasts, DVE recip+mul."""
            denrows = []
            for h in range(2):
                denrow = pool_r.tile([65, 512], F32, tag="denrow")
                nc.vector.tensor_copy(
                    denrow[64:65, :], avden[64:65, h * 512:(h + 1) * 512])
                denrows.append(denrow)
            bcs = []
            for h in range(2):
                bc_ps = ps_mm.tile([128, 512], F32, tag="mm")
                nc.tensor.matmul(
                    bc_ps[0:64, :], lhsT=ones32[64:65, :], rhs=denrows[h][64:65, :],
                    start=True, stop=True,
                )
                bcs.append(bc_ps)
            for h in range(2):
                recip = pool_r.tile([64, 512], F32, tag="recip")
                nc.vector.reciprocal_approx_fast(recip[:], bcs[h][0:64, :])
                nc.vector.tensor_mul(
                    att_sb[h * 64:(h + 1) * 64, j * 512:(j + 1) * 512],
                    avden[0:64, h * 512:(h + 1) * 512], recip[:],
                )

        # warm-up stream, then round 0 projections (PE hot when xT lands)
        warm_ps = ps_s.tile([128, 1024], F32, tag="s")
        for _ in range(WARM_N):
            nc.tensor.matmul(
                warm_ps[:, 0:512], lhsT=warm_sb[:, 0:128], rhs=warm_sb[:, 128:640],
                start=True, stop=True,
            )
        for _ in proj_gen(0):
            pass

        # r=0: attn(0) ⟂ proj(1); normalize(0); proj(1) rest; oproj(0) t 0-2
        # r=1: attn(1) ⟂ proj(2); normalize(1); proj(2) rest; oproj(1) t 4-6
        # r=2: attn(2) ⟂ proj(3); normalize(2); proj(3) rest
        # r=3: attn(3) ⟂ oproj(2) t 8-10; normalize(3); deferred t 3,7,11
        #      (ready att data keeps the PE busy under normalize(3)); oproj(3)
        quotas = (12, 6, 4, 1)
        work = _Work(proj_gen(1))
        for r in range(NT512):
            avden = attn_round(r, work, quotas[r])
            work.drain_to_boundary()
            if r == 3:
                # deferred tiles have long-ready att data: they keep the PE
                # busy while normalize(3)'s DVE chain runs
                _Work(oproj_gen([3, 7])).drain_all()
            normalize(r, avden)
            work.drain_all()
            if r < 2:
                _Work(oproj_gen([4 * r, 4 * r + 1, 4 * r + 2])).drain_all()
                work = _Work(proj_gen(r + 2))
            elif r == 2:
                work = _Work(oproj_gen([8, 9, 10]))
            else:
                _Work(oproj_gen([11, 12, 13, 14, 15])).drain_all()

        if dbg is not None:
            for name, sb in (("qT", qT_sb), ("kT", kT_sb), ("att", att_sb)):
                nc.sync.dma_start(dbg[name][:], sb[:])


def _build_program(debug_dumps=False):
    nc = bacc.Bacc("TRN2", debug=False, num_devices=N_CORES)
    xT = nc.dram_tensor("xT", [D, T], BF, kind="ExternalInput").ap()
    wq = nc.dram_tensor("wq", [128, D], BF, kind="ExternalInput").ap()
    wk = nc.dram_tensor("wk", [128, D], BF, kind="ExternalInput").ap()
    wv = nc.dram_tensor("wv", [128, D], BF, kind="ExternalInput").ap()
    wo = nc.dram_tensor("wo", [128, D], BF, kind="ExternalInput").ap()
    mask = nc.dram_tensor("mask", [128, 128], BF, kind="ExternalInput").ap()
    y = nc.dram_tensor("y", [T, D], BF, kind="ExternalOutput").ap()
    dbg = None
    if debug_dumps:
        dbg = {
            name: nc.dram_tensor(f"dbg_{name}", [128, T], BF, kind="ExternalOutput").ap()
            for name in ("qT", "kT", "att")
        }

    with tile.TileContext(nc) as tc:
        _kernel(tc, y, xT, wq, wk, wv, wo, mask, dbg=dbg)
    nc.compile()
    return nc


_NC = None


def _get_program():
    global _NC
    if _NC is None:
        _NC = _build_program()
    return _NC


def _rearrange_w(w_cols):
    """[1024, 128] f32 slice of W_qkv -> [128, 1024] bf16 with d-chunk d at
    cols [d*128, (d+1)*128): out[p, d*128 + m] = w_cols[d*128 + p, m]."""
    return np.ascontiguousarray(
        w_cols.reshape(KD, 128, 128).transpose(1, 0, 2).reshape(128, KD * 128)
    ).astype(BF16)


def make_in_maps(x, W_qkv, W_o):
    x2 = np.asarray(x, dtype=np.float32).reshape(T, D)
    W_qkv = np.asarray(W_qkv, dtype=np.float32)
    W_o = np.asarray(W_o, dtype=np.float32)

    xT_bf = np.ascontiguousarray(x2.T).astype(BF16)
    mask = np.triu(np.ones((128, 128), dtype=np.float32)).astype(BF16)

    in_maps = []
    for c in range(N_CORES):
        cs = slice(2 * c * HD, 2 * c * HD + 128)
        in_maps.append({
            "xT": xT_bf,
            "wq": _rearrange_w(W_qkv[:, 0 * D:1 * D][:, cs]),
            "wk": _rearrange_w(W_qkv[:, 1 * D:2 * D][:, cs]),
            "wv": _rearrange_w(W_qkv[:, 2 * D:3 * D][:, cs]),
            "wo": np.ascontiguousarray(W_o[c * 128:(c + 1) * 128, :]).astype(BF16),
            "mask": mask,
        })
    return in_maps


def combine_outputs(results):
    y_full = np.zeros((T, D), dtype=np.float32)
    for c in range(N_CORES):
        y_full += results[c]["y"].astype(np.float32)
    return y_full.reshape(1, T, D)


def kernel(x, W_qkv, W_o):
    from concourse.bass_utils import run_bass_kernel_spmd

    nc = _get_program()
    in_maps = make_in_maps(x, W_qkv, W_o)
    res = run_bass_kernel_spmd(nc, in_maps, core_ids=list(range(N_CORES)))
    return combine_outputs(res.results)
